# revision 7
# baseline (speedup 1.0000x reference)
"""BertLayer forward on 8 Trainium2 NeuronCores.

Sharding: token-parallel. The B*S = 4096 tokens are split 512/core (4 cores
per batch element). Each core recomputes its batch's full QKV projection
locally (attention needs all keys/values of the batch), so there is no
inter-core communication at all. All activations are kept channel-major
(channels on SBUF partitions, tokens on the free dim) so every matmul in the
chain is `w_blockT.T @ xT` with contraction on the partition dim.

Numerics: all matmuls run in float32r (fp32 storage, ~2^-13 effective matmul
mantissa, 1 cycle/row on the PE — 4x faster than plain fp32).

Tricks:
  - attention mask is folded into the scores matmul as a 65th contraction row
    (kT_aug row 64 = 8*mask[t], qT_aug row 64 = ones), so exp() needs no
    per-t-tile bias and can run over 3-bank PSUM groups.
  - softmax denominators come for free as a 65th output row of the
    probs.T @ v_aug matmul (v_aug column 64 = ones).
  - LayerNorm channel-dim reductions are ones-vector matmuls on the PE;
    per-token mean/rstd rows are partition-broadcast by the GPSIMD engine.
"""
import numpy as np
from contextlib import ExitStack

B, S, D = 2, 2048, 1024
H, DH = 16, 64
DFF = 4096
EPS = 1e-5
NCORES = 8
TOK = (B * S) // NCORES          # 512 tokens owned per core
CPB = NCORES // B                # 4 cores per batch
CH_T = D // 128                  # 8 channel tiles
DFF_T = DFF // 128               # 32 dff tiles
T_T = S // 128                   # 16 key-token tiles

_CACHE = {}


def _build(sim_tanh_gelu=False, dbg=False):
    import concourse.bass as bass
    import concourse.tile as tile
    from concourse import bacc, mybir
    from concourse.masks import make_identity

    F32 = mybir.dt.float32
    F32R = mybir.dt.float32r
    AF = mybir.ActivationFunctionType
    OP = mybir.AluOpType

    nc = bacc.Bacc("TRN2", target_bir_lowering=False, debug=False,
                   num_devices=NCORES)

    h_t = nc.dram_tensor("h_t", [D, S], F32R, kind="ExternalInput").ap()
    h_own = nc.dram_tensor("h_own", [D, TOK], F32, kind="ExternalInput").ap()
    mask8 = nc.dram_tensor("mask8", [1, S], F32R, kind="ExternalInput").ap()
    wq = nc.dram_tensor("wq", [128, CH_T * D], F32R, kind="ExternalInput").ap()
    wso = nc.dram_tensor("wso", [128, CH_T * D], F32R, kind="ExternalInput").ap()
    wi = nc.dram_tensor("wi", [128, DFF_T * D], F32R, kind="ExternalInput").ap()
    wo = nc.dram_tensor("wo", [128, CH_T * DFF], F32R, kind="ExternalInput").ap()
    qb = nc.dram_tensor("qb", [128, CH_T], F32, kind="ExternalInput").ap()
    sob = nc.dram_tensor("sob", [128, CH_T], F32, kind="ExternalInput").ap()
    ib = nc.dram_tensor("ib", [128, DFF_T], F32, kind="ExternalInput").ap()
    ob = nc.dram_tensor("ob", [128, CH_T], F32, kind="ExternalInput").ap()
    l1g = nc.dram_tensor("l1g", [128, CH_T], F32, kind="ExternalInput").ap()
    l1b = nc.dram_tensor("l1b", [128, CH_T], F32, kind="ExternalInput").ap()
    l2g = nc.dram_tensor("l2g", [128, CH_T], F32, kind="ExternalInput").ap()
    l2b = nc.dram_tensor("l2b", [128, CH_T], F32, kind="ExternalInput").ap()
    out = nc.dram_tensor("out", [TOK, D], F32, kind="ExternalOutput").ap()
    dbg_aps = {}
    if dbg:
        for nm, sh in (("d_qkvT", [128, CH_T * S]), ("d_attnT", [128, CH_T * TOK]),
                       ("d_x", [128, CH_T * TOK]), ("d_xln", [128, CH_T * TOK]),
                       ("d_g", [128, DFF_T * TOK]), ("d_z", [128, CH_T * TOK]),
                       ("d_rb", [128, TOK]), ("d_mb", [128, TOK]),
                       ("d_mu", [1, TOK]), ("d_rstd", [1, TOK])):
            dbg_aps[nm] = nc.dram_tensor(nm, sh, F32, kind="ExternalOutput").ap()

    with tile.TileContext(nc) as tc, ExitStack() as root:
        const = root.enter_context(tc.tile_pool(name="const", bufs=1))
        ones2_f = const.tile([128, 2], F32, tag="ones2f")
        nc.vector.memset(ones2_f[:], 1.0)
        ones_col = const.tile([128, 1], F32R, tag="onescol")
        nc.vector.tensor_copy(ones_col[:], ones2_f[:, 0:1])
        ones_row = const.tile([1, TOK], F32, tag="onesrowf")
        nc.vector.memset(ones_row[:], 1.0)
        ident_f = const.tile([128, 128], F32, tag="identf")
        make_identity(nc, ident_f[:])
        ident_r = const.tile([128, 128], F32R, tag="identr")
        nc.vector.tensor_copy(ident_r[:], ident_f[:])

        bias_p = root.enter_context(tc.tile_pool(name="bias", bufs=1))
        qb_s = bias_p.tile([128, CH_T], F32, tag="qb")
        sob_s = bias_p.tile([128, CH_T], F32, tag="sob")
        ib_s = bias_p.tile([128, DFF_T], F32, tag="ib")
        ob_s = bias_p.tile([128, CH_T], F32, tag="ob")
        l1g_s = bias_p.tile([128, CH_T], F32, tag="l1g")
        l1b_s = bias_p.tile([128, CH_T], F32, tag="l1b")
        l2g_s = bias_p.tile([128, CH_T], F32, tag="l2g")
        l2b_s = bias_p.tile([128, CH_T], F32, tag="l2b")
        for t, a in ((qb_s, qb), (sob_s, sob), (ib_s, ib), (ob_s, ob),
                     (l1g_s, l1g), (l1b_s, l1b), (l2g_s, l2g), (l2b_s, l2b)):
            nc.sync.dma_start(t[:], a[:])

        # DVE scratch shared by LN phases
        scr = root.enter_context(tc.tile_pool(name="scratch", bufs=2))

        # long-lived activation tensors, opened in LIFO-compatible order
        xln_scope = ExitStack()
        xlnp = xln_scope.enter_context(tc.tile_pool(name="xln", bufs=1))
        xln = xlnp.tile([128, CH_T * TOK], F32R, tag="xln")

        attn_scope = ExitStack()
        attnp = attn_scope.enter_context(tc.tile_pool(name="attn", bufs=1))
        attnT = attnp.tile([128, CH_T * TOK], F32R, tag="attnT")

        qkv_scope = ExitStack()
        qkvp = qkv_scope.enter_context(tc.tile_pool(name="qkvT", bufs=1))
        qkvT = qkvp.tile([128, CH_T * S], F32R, tag="qkvT")

        # ---------------- Phase 1: qkvT = wq @ h_t  (full batch) ----------
        with tc.tile_pool(name="wq_p", bufs=1) as wq_p, \
             tc.tile_pool(name="ht_p", bufs=2) as ht_p, \
             tc.tile_pool(name="ps_qkv", bufs=3, space="PSUM") as ps_qkv:
            wq_s = wq_p.tile([128, CH_T * D], F32R, tag="wq")
            nc.sync.dma_start(wq_s[:], wq[:])
            for n in range(S // 512):
                ht = []
                for k in range(CH_T):
                    t = ht_p.tile([128, 512], F32R, tag=f"ht{k}")
                    nc.sync.dma_start(
                        t[:], h_t[k * 128:(k + 1) * 128, n * 512:(n + 1) * 512])
                    ht.append(t)
                for m in range(CH_T):
                    ps = ps_qkv.tile([128, 512], F32, tag="ps")
                    for k in range(CH_T):
                        nc.tensor.matmul(
                            ps[:], wq_s[:, m * D + k * 128:m * D + k * 128 + 128],
                            ht[k][:], start=(k == 0), stop=(k == CH_T - 1))
                    nc.vector.tensor_scalar_add(
                        qkvT[:, m * S + n * 512:m * S + (n + 1) * 512],
                        ps[:], qb_s[:, m:m + 1])

        if dbg:
            nc.sync.dma_start(dbg_aps["d_qkvT"][:], qkvT[:].bitcast(F32))
        # ---------------- Phase 2: attention ------------------------------
        GROUPS = [(0, 3), (3, 3), (6, 3), (9, 3), (12, 2), (14, 2)]
        with tc.tile_pool(name="vA_p", bufs=2) as vA_p, \
             tc.tile_pool(name="ktaug_p", bufs=2) as kt_p, \
             tc.tile_pool(name="qtaug_p", bufs=2) as qt_p, \
             tc.tile_pool(name="ps_tr", bufs=1, space="PSUM") as ps_tr, \
             tc.tile_pool(name="ps_sc", bufs=2, space="PSUM") as ps_sc, \
             tc.tile_pool(name="ps_at", bufs=1, space="PSUM") as ps_at, \
             tc.tile_pool(name="probs_p", bufs=3) as probs_p, \
             tc.tile_pool(name="rec_p", bufs=2) as rec_p:
            for m in range(CH_T):
                # v for heads 2m, 2m+1: transpose qkvT chunk to token-major,
                # interleave a ones column per head for the softmax denom.
                vA = vA_p.tile([128, T_T * 130], F32R, tag="vA")
                for i in range(T_T):
                    pt = ps_tr.tile([128, 128], F32R, tag="pt")
                    nc.tensor.transpose(
                        pt[:], qkvT[:, m * S + i * 128:m * S + (i + 1) * 128],
                        ident_r[:])
                    dst = vA[:, i * 130:(i + 1) * 130].rearrange(
                        "p (g c) -> p g c", c=65)[:, :, 0:64]
                    src = pt[:].rearrange("p (g c) -> p g c", g=2)
                    nc.vector.tensor_copy(dst, src)
                    ones_dst = vA[:, i * 130:(i + 1) * 130].rearrange(
                        "p (g c) -> p g c", c=65)[:, :, 64:65]
                    nc.vector.tensor_copy(
                        ones_dst, ones2_f[:].rearrange("p (g c) -> p g c", c=1))
                for sub in range(2):
                    h0 = sub * 64
                    ktaug = kt_p.tile([65, S], F32R, tag="ktaug")
                    nc.vector.tensor_copy(
                        ktaug[0:64, :], qkvT[h0:h0 + 64, m * S:(m + 1) * S])
                    nc.sync.dma_start(ktaug[64:65, :], mask8[:])
                    qtaug = qt_p.tile([65, TOK], F32R, tag="qtaug")
                    nc.vector.tensor_copy(
                        qtaug[0:64, :], qkvT[h0:h0 + 64, m * S:m * S + TOK])
                    nc.vector.tensor_copy(qtaug[64:65, :], ones_row[:])

                    pat = ps_at.tile([65, TOK], F32, tag="pat")
                    for g0, glen in GROUPS:
                        psc = ps_sc.tile([128, 3 * 512], F32, tag="psc")
                        for j in range(glen):
                            i = g0 + j
                            nc.tensor.matmul(
                                psc[:, j * 512:(j + 1) * 512],
                                ktaug[:, i * 128:(i + 1) * 128], qtaug[:],
                                start=True, stop=True)
                        probs = probs_p.tile([128, 3 * 512], F32R, tag="probs")
                        nc.scalar.activation(
                            probs[:, 0:glen * 512], psc[:, 0:glen * 512],
                            AF.Exp, scale=float(1.0 / np.sqrt(DH)))
                        for j in range(glen):
                            i = g0 + j
                            nc.tensor.matmul(
                                pat[:],
                                vA[:, i * 130 + sub * 65:i * 130 + sub * 65 + 65],
                                probs[:, j * 512:(j + 1) * 512],
                                start=(i == 0), stop=(i == T_T - 1))
                    rec = rec_p.tile([1, TOK], F32, tag="rec")
                    nc.vector.reciprocal(rec[:], pat[64:65, :])
                    recb = rec_p.tile([64, TOK], F32, tag="recb")
                    nc.gpsimd.partition_broadcast(recb[:], rec[:])
                    nc.vector.tensor_mul(
                        attnT[h0:h0 + 64, m * TOK:(m + 1) * TOK],
                        pat[0:64, :], recb[:])
        if dbg:
            nc.sync.dma_start(dbg_aps["d_attnT"][:], attnT[:].bitcast(F32))
        qkv_scope.close()

        # ---------------- Phase 3: self-output + LN1 ----------------------
        with tc.tile_pool(name="wso_p", bufs=3) as wso_p, \
             tc.tile_pool(name="hown_p", bufs=1) as hown_p, \
             tc.tile_pool(name="x_p", bufs=1) as x_p, \
             tc.tile_pool(name="ps_so", bufs=3, space="PSUM") as ps_so, \
             tc.tile_pool(name="ps_sum", bufs=1, space="PSUM") as ps_sum, \
             tc.tile_pool(name="ln_small", bufs=1) as lnp, \
             tc.tile_pool(name="lnb_p", bufs=1) as lnb_p:
            hown_s = hown_p.tile([128, CH_T * TOK], F32, tag="hown")
            for m in range(CH_T):
                nc.sync.dma_start(
                    hown_s[:, m * TOK:(m + 1) * TOK],
                    h_own[m * 128:(m + 1) * 128, :])
            x_sb = x_p.tile([128, CH_T * TOK], F32R, tag="x")
            pss = ps_sum.tile([1, TOK], F32, tag="s")
            psq = ps_sum.tile([1, TOK], F32, tag="q")
            for m in range(CH_T):
                wsom = wso_p.tile([128, D], F32R, tag="wsom")
                nc.sync.dma_start(wsom[:], wso[:, m * D:(m + 1) * D])
                ps = ps_so.tile([128, TOK], F32, tag="ps")
                for k in range(CH_T):
                    nc.tensor.matmul(
                        ps[:], wsom[:, k * 128:(k + 1) * 128],
                        attnT[:, k * TOK:(k + 1) * TOK],
                        start=(k == 0), stop=(k == CH_T - 1))
                xs = x_sb[:, m * TOK:(m + 1) * TOK]
                nc.vector.scalar_tensor_tensor(
                    xs, ps[:], sob_s[:, m:m + 1],
                    hown_s[:, m * TOK:(m + 1) * TOK], OP.add, OP.add)
                sq = scr.tile([128, TOK], F32R, tag="sq")
                nc.vector.tensor_mul(sq[:], xs, xs)
                nc.tensor.matmul(pss[:], ones_col[:], xs,
                                 start=(m == 0), stop=(m == CH_T - 1))
                nc.tensor.matmul(psq[:], ones_col[:], sq[:],
                                 start=(m == 0), stop=(m == CH_T - 1))

            mu = lnp.tile([1, TOK], F32, tag="mu1")
            ex2 = lnp.tile([1, TOK], F32, tag="ex21")
            nc.scalar.mul(mu[:], pss[:], 1.0 / D)
            nc.scalar.mul(ex2[:], psq[:], 1.0 / D)
            sqmu = lnp.tile([1, TOK], F32, tag="sqmu1")
            nc.vector.tensor_mul(sqmu[:], mu[:], mu[:])
            vare = lnp.tile([1, TOK], F32, tag="vare1")
            nc.vector.scalar_tensor_tensor(vare[:], ex2[:], EPS, sqmu[:],
                                           OP.add, OP.subtract)
            rcp = lnp.tile([1, TOK], F32, tag="rcp1")
            nc.vector.reciprocal(rcp[:], vare[:])
            rstd = lnp.tile([1, TOK], F32, tag="rstd1")
            nc.scalar.sqrt(rstd[:], rcp[:])
            rstd_b = lnb_p.tile([128, TOK], F32, tag="rstdb1")
            mu_b = lnb_p.tile([128, TOK], F32, tag="mub1")
            nc.gpsimd.partition_broadcast(rstd_b[:], rstd[:])
            nc.gpsimd.partition_broadcast(mu_b[:], mu[:])
            if dbg:
                nc.sync.dma_start(dbg_aps["d_rb"][:], rstd_b[:])
                nc.sync.dma_start(dbg_aps["d_mb"][:], mu_b[:])
                nc.sync.dma_start(dbg_aps["d_mu"][:], mu[:])
                nc.sync.dma_start(dbg_aps["d_rstd"][:], rstd[:])
            for m in range(CH_T):
                xs = x_sb[:, m * TOK:(m + 1) * TOK]
                d = scr.tile([128, TOK], F32, tag="d")
                nc.vector.tensor_sub(d[:], xs, mu_b[:])
                e = scr.tile([128, TOK], F32, tag="e")
                nc.vector.scalar_tensor_tensor(
                    e[:], d[:], l1g_s[:, m:m + 1], rstd_b[:], OP.mult, OP.mult)
                nc.vector.tensor_scalar_add(
                    xln[:, m * TOK:(m + 1) * TOK], e[:], l1b_s[:, m:m + 1])
            if dbg:
                nc.sync.dma_start(dbg_aps["d_x"][:], x_sb[:].bitcast(F32))
                nc.sync.dma_start(dbg_aps["d_xln"][:], xln[:].bitcast(F32))
        attn_scope.close()

        # ---------------- Phase 4: FFN1 + GELU ----------------------------
        g_scope = ExitStack()
        gp = g_scope.enter_context(tc.tile_pool(name="g_p", bufs=1))
        g_sb = gp.tile([128, DFF_T * TOK], F32R, tag="g")
        with tc.tile_pool(name="wi_p", bufs=3) as wi_p, \
             tc.tile_pool(name="ps_f1", bufs=3, space="PSUM") as ps_f1:
            for m in range(DFF_T):
                wim = wi_p.tile([128, D], F32R, tag="wim")
                nc.sync.dma_start(wim[:], wi[:, m * D:(m + 1) * D])
                ps = ps_f1.tile([128, TOK], F32, tag="ps")
                for k in range(CH_T):
                    nc.tensor.matmul(
                        ps[:], wim[:, k * 128:(k + 1) * 128],
                        xln[:, k * TOK:(k + 1) * TOK],
                        start=(k == 0), stop=(k == CH_T - 1))
                nc.scalar.activation(
                    g_sb[:, m * TOK:(m + 1) * TOK], ps[:],
                    AF.Tanh if sim_tanh_gelu else AF.Gelu,
                    bias=ib_s[:, m:m + 1])

        if dbg:
            nc.sync.dma_start(dbg_aps["d_g"][:], g_sb[:].bitcast(F32))
        # ---------------- Phase 5: FFN2 + LN2 + transpose out -------------
        with tc.tile_pool(name="wo_p", bufs=2) as wo_p, \
             tc.tile_pool(name="ps_f2", bufs=3, space="PSUM") as ps_f2, \
             tc.tile_pool(name="z_p", bufs=1) as z_p, \
             tc.tile_pool(name="ps_sum2", bufs=1, space="PSUM") as ps_sum2, \
             tc.tile_pool(name="ln2_small", bufs=1) as ln2p, \
             tc.tile_pool(name="ln2b_p", bufs=1) as ln2b_p, \
             tc.tile_pool(name="y_p", bufs=2) as y_p, \
             tc.tile_pool(name="ps_otr", bufs=2, space="PSUM") as ps_otr, \
             tc.tile_pool(name="stage_p", bufs=1) as stage_p:
            z_sb = z_p.tile([128, CH_T * TOK], F32R, tag="z")
            pss2 = ps_sum2.tile([1, TOK], F32, tag="s")
            psq2 = ps_sum2.tile([1, TOK], F32, tag="q")
            for m in range(CH_T):
                wom = wo_p.tile([128, DFF], F32R, tag="wom")
                nc.sync.dma_start(wom[:], wo[:, m * DFF:(m + 1) * DFF])
                ps = ps_f2.tile([128, TOK], F32, tag="ps")
                for k in range(DFF_T):
                    nc.tensor.matmul(
                        ps[:], wom[:, k * 128:(k + 1) * 128],
                        g_sb[:, k * TOK:(k + 1) * TOK],
                        start=(k == 0), stop=(k == DFF_T - 1))
                zs = z_sb[:, m * TOK:(m + 1) * TOK]
                nc.vector.scalar_tensor_tensor(
                    zs, ps[:], ob_s[:, m:m + 1],
                    xln[:, m * TOK:(m + 1) * TOK], OP.add, OP.add)
                sq = scr.tile([128, TOK], F32R, tag="sq")
                nc.vector.tensor_mul(sq[:], zs, zs)
                nc.tensor.matmul(pss2[:], ones_col[:], zs,
                                 start=(m == 0), stop=(m == CH_T - 1))
                nc.tensor.matmul(psq2[:], ones_col[:], sq[:],
                                 start=(m == 0), stop=(m == CH_T - 1))

            mu2 = ln2p.tile([1, TOK], F32, tag="mu2")
            ex22 = ln2p.tile([1, TOK], F32, tag="ex22")
            nc.scalar.mul(mu2[:], pss2[:], 1.0 / D)
            nc.scalar.mul(ex22[:], psq2[:], 1.0 / D)
            sqmu2 = ln2p.tile([1, TOK], F32, tag="sqmu2")
            nc.vector.tensor_mul(sqmu2[:], mu2[:], mu2[:])
            vare2 = ln2p.tile([1, TOK], F32, tag="vare2")
            nc.vector.scalar_tensor_tensor(vare2[:], ex22[:], EPS, sqmu2[:],
                                           OP.add, OP.subtract)
            rcp2 = ln2p.tile([1, TOK], F32, tag="rcp2")
            nc.vector.reciprocal(rcp2[:], vare2[:])
            rstd2 = ln2p.tile([1, TOK], F32, tag="rstd2")
            nc.scalar.sqrt(rstd2[:], rcp2[:])
            rstd2_b = ln2b_p.tile([128, TOK], F32, tag="rstdb2")
            mu2_b = ln2b_p.tile([128, TOK], F32, tag="mub2")
            nc.gpsimd.partition_broadcast(rstd2_b[:], rstd2[:])
            nc.gpsimd.partition_broadcast(mu2_b[:], mu2[:])

            if dbg:
                nc.sync.dma_start(dbg_aps["d_z"][:], z_sb[:].bitcast(F32))
            stage = stage_p.tile([128, (TOK // 128) * D], F32, tag="stage")
            for m in range(CH_T):
                zs = z_sb[:, m * TOK:(m + 1) * TOK]
                d = scr.tile([128, TOK], F32, tag="d")
                nc.vector.tensor_sub(d[:], zs, mu2_b[:])
                e = scr.tile([128, TOK], F32, tag="e")
                nc.vector.scalar_tensor_tensor(
                    e[:], d[:], l2g_s[:, m:m + 1], rstd2_b[:], OP.mult, OP.mult)
                y_m = y_p.tile([128, TOK], F32, tag="y")
                nc.vector.tensor_scalar_add(y_m[:], e[:], l2b_s[:, m:m + 1])
                for j in range(TOK // 128):
                    pt = ps_otr.tile([128, 128], F32, tag="pt")
                    nc.tensor.transpose(
                        pt[:], y_m[:, j * 128:(j + 1) * 128], ident_f[:])
                    nc.scalar.copy(
                        stage[:, j * D + m * 128:j * D + (m + 1) * 128], pt[:])
            for j in range(TOK // 128):
                nc.sync.dma_start(out[j * 128:(j + 1) * 128, :],
                                  stage[:, j * D:(j + 1) * D])
        g_scope.close()
        xln_scope.close()
    nc.finalize()
    return nc


def _blockify(wt, kt, mt):
    # wt: [kt*128, mt*128] (already W.T). Block (m, k) lands at columns
    # [m*kt*128 + k*128, ...+128) so a per-m slab is one contiguous DMA.
    return np.ascontiguousarray(
        wt.reshape(kt, 128, mt, 128).transpose(1, 2, 0, 3).reshape(128, -1))


def _cols(bias, nt):
    return np.ascontiguousarray(np.asarray(bias, np.float32).reshape(nt, 128).T)


def kernel(hidden_state, attention_mask, q_w, q_b, so_w, so_b, ln1_g, ln1_b,
           inter_w, inter_b, out_w, out_b, ln2_g, ln2_b):
    from concourse.bass_utils import run_bass_kernel_spmd

    if "nc" not in _CACHE:
        _CACHE["nc"] = _build()
    nc = _CACHE["nc"]

    hidden_state = np.asarray(hidden_state, np.float32)
    attention_mask = np.asarray(attention_mask, np.float32)

    shared = {
        "wq": _blockify(np.asarray(q_w, np.float32).T, CH_T, CH_T),
        "wso": _blockify(np.asarray(so_w, np.float32).T, CH_T, CH_T),
        "wi": _blockify(np.asarray(inter_w, np.float32).T, CH_T, DFF_T),
        "wo": _blockify(np.asarray(out_w, np.float32).T, DFF_T, CH_T),
        "qb": _cols(q_b, CH_T), "sob": _cols(so_b, CH_T),
        "ib": _cols(inter_b, DFF_T), "ob": _cols(out_b, CH_T),
        "l1g": _cols(ln1_g, CH_T), "l1b": _cols(ln1_b, CH_T),
        "l2g": _cols(ln2_g, CH_T), "l2b": _cols(ln2_b, CH_T),
    }
    in_maps = []
    for c in range(NCORES):
        b, r = divmod(c, CPB)
        ht = np.ascontiguousarray(hidden_state[b].T)         # [D, S]
        ht_rot = np.roll(ht, -r * TOK, axis=1)               # own tokens first
        m8 = np.roll(8.0 * attention_mask[b, 0, 0, :], -r * TOK).reshape(1, S)
        in_maps.append({
            **shared,
            "h_t": np.ascontiguousarray(ht_rot),
            "h_own": np.ascontiguousarray(ht[:, r * TOK:(r + 1) * TOK]),
            "mask8": np.ascontiguousarray(m8.astype(np.float32)),
        })

    res = run_bass_kernel_spmd(nc, in_maps, list(range(NCORES)))
    full = np.empty((B, S, D), np.float32)
    for c in range(NCORES):
        b, r = divmod(c, CPB)
        full[b, r * TOK:(r + 1) * TOK, :] = res.results[c]["out"]
    return full
